# revision 38
# baseline (speedup 1.0000x reference)
"""Trainium2 Bass kernel for nn_Bottleneck_75325136437765 (sparse 3x3 local attention bottleneck).

Sharding: data-parallel over batch B=16 across 8 cores (2 batches/core), params replicated.

Per-core layout: channels on partitions, spatial (32*32=1024) on free dim. All matmuls bf16
(fp32 matmuls cost 2 PE passes on TRN2), fp32 PSUM accumulation everywhere.

  conv1/qkv/conv3: plain matmuls (lhsT = transposed weights, host-precomputed, bn scales folded).
  attention logits, packed PSUM layout (row = 32*(kk%4) + head, 3 tiles of 4 shifts):
      L[g,kk,hw] = sum_d q[gd,hw]*k[gd,hw+off_kk]  (col-tiled 0/1-selection matmuls over products)
                 + sum_d q[gd,hw]*pos[gd,kk]       (P2 matmul, accumulated into same PSUM)
  softmax over kk without max-subtraction, 1/sum factored out to the end:
      e = exp(L) (packed, 3 ACT ops); den = sum_kk e via 0/1 matmuls; recip = 1/den
      out_pre[c,hw] = sum_kk e_bc[c,kk,hw] * v[c,hw+off_kk]
        e_bc via row-tiled expansion matmuls; per-shift product on DVE;
        sum over kk via identity-matmul PSUM accumulation
      h2 = relu(out_pre * recip_bc + bnatt_b)
  residual: x streamed again in fp32, fused (x+b3)+psum on DVE, relu on ACT.
"""

import numpy as np

import concourse.bass as bass
import concourse.bacc as bacc
import concourse.tile as tile
from concourse import mybir
from concourse.bass_utils import run_bass_kernel_spmd

# ---- problem constants (hardcoded per contract) ----
B, CIN, H, W = 16, 1024, 32, 32
WIDTH, OUT, HEADS, KS = 256, 1024, 32, 3
D = WIDTH // HEADS            # 8 channels per head
HW = H * W                    # 1024
NC_ = 8                       # cores
BL = B // NC_                 # 2 batches per core
P = 128
KC1 = CIN // P                # 8 contraction chunks for conv1
PT = WIDTH // P               # 2 partition tiles for width-256 tensors
OC = OUT // P                 # 8 output ptiles for conv3
NKK = KS * KS                 # 9 shifts
NT = 3                        # packed logit tiles (4+4+1 shifts)
F32 = mybir.dt.float32
BF16 = mybir.dt.bfloat16
NHALF = 2                     # PSUM-bank limit: matmul N<=512 fp32 out


def _ns(n):
    return slice(n * 512, (n + 1) * 512)


def build_program():
    nc = bacc.Bacc(None, target_bir_lowering=False, debug=False)

    def din(name, shape, dt=BF16):
        return nc.dram_tensor(name, list(shape), dt, kind="ExternalInput").ap()

    x16_d = din("x16", (BL, KC1, P, HW))
    w1T_d = din("w1T", (KC1, P, WIDTH))
    wqT_d = din("wqT", (PT, P, WIDTH))
    wkT_d = din("wkT", (PT, P, WIDTH))
    wvT_d = din("wvT", (PT, P, WIDTH))
    w3T_d = din("w3T", (PT, P, OUT))
    b1_d = din("b1", (PT, P, 1), F32)
    bq_d = din("bq", (PT, P, 1), F32)
    bk_d = din("bk", (PT, P, 1), F32)
    bv_d = din("bv", (PT, P, 1), F32)
    batt_d = din("batt", (PT, P, 1), F32)
    b3_d = din("b3", (OC, P, 1), F32)
    sel_d = din("sel", (PT, P, HEADS))
    p2_d = din("p2", (PT, P, NT, P))
    sab_d = din("sab", (P, HEADS))
    eye32_d = din("eye32", (HEADS, HEADS))
    ident_d = din("ident", (P, P))
    out_d = nc.dram_tensor("out", [BL, OC, P, HW], F32, kind="ExternalOutput").ap()

    with tile.TileContext(nc) as tc:
        with (
            tc.tile_pool(name="consts", bufs=1) as consts,
            tc.tile_pool(name="xb", bufs=2) as xbp,
            tc.tile_pool(name="act", bufs=2) as actp,
            tc.tile_pool(name="attn", bufs=1) as attnp,
            tc.tile_pool(name="epk", bufs=4) as epkp,
            tc.tile_pool(name="tmp", bufs=10) as tmpp,
            tc.tile_pool(name="tmp2", bufs=4) as tmp2p,
            tc.tile_pool(name="ebc", bufs=9) as ebcp,
            tc.tile_pool(name="outz", bufs=3) as outzp,
            tc.tile_pool(name="pmm", bufs=2, space="PSUM") as pmm,
            tc.tile_pool(name="pL", bufs=1, space="PSUM") as pLp,
            tc.tile_pool(name="pacc", bufs=1, space="PSUM") as paccp,
        ):
            # ---- load constants ----
            # constants other than w1T/b1 go on the SWDGE queue so the sync
            # queue serves conv1's x/w chunks first (fast kernel start)
            def cload(name, dram, shape, dt=BF16, re="k p m -> p k m"):
                t = consts.tile(shape, dt, tag=name)
                nc.gpsimd.dma_start(out=t, in_=dram.rearrange(re) if re else dram)
                return t

            w1T = consts.tile([P, KC1, WIDTH], BF16, tag="w1T")
            b1 = consts.tile([P, PT, 1], F32, tag="b1")
            nc.sync.dma_start(out=b1, in_=b1_d.rearrange("k p m -> p k m"))
            wqT = cload("wqT", wqT_d, [P, PT, WIDTH])
            wkT = cload("wkT", wkT_d, [P, PT, WIDTH])
            wvT = cload("wvT", wvT_d, [P, PT, WIDTH])
            w3T = cload("w3T", w3T_d, [P, PT, OUT])
            bq = cload("bq", bq_d, [P, PT, 1], F32)
            bk = cload("bk", bk_d, [P, PT, 1], F32)
            bv = cload("bv", bv_d, [P, PT, 1], F32)
            batt = cload("batt", batt_d, [P, PT, 1], F32)
            b3 = cload("b3", b3_d, [P, OC, 1], F32)
            sel = cload("sel", sel_d, [P, PT, HEADS])
            p2 = cload("p2", p2_d, [P, PT, NT, P], re="k p m o -> p k m o")
            sab = cload("sab", sab_d, [P, HEADS], re=None)
            eye32 = cload("eye32", eye32_d, [HEADS, HEADS], re=None)
            ident = cload("ident", ident_d, [P, P], re=None)

            def head_bcast_dma(dst, src16):
                # dst[g*8+d, :] = src16[g, :] — 2-level partition AP broadcast
                bc = bass.AP(tensor=src16.tensor, offset=src16.offset,
                             ap=[list(src16.ap[0]), [0, D]]
                                + [list(a) for a in src16.ap[1:]])
                nc.sync.dma_start(out=dst, in_=bc)

            # persistent zero-padded k/v tiles (borders stay zero across batches)
            kpad = consts.tile([P, PT, H + 2, W + 2], BF16)
            vpad = consts.tile([P, PT, H + 2, W + 2], BF16)
            nc.vector.memset(kpad, 0.0)
            nc.vector.memset(vpad, 0.0)

            for b in range(BL):
                # ---- load x (bf16 for conv1), chunked so conv1 starts early ----
                xb = xbp.tile([P, KC1, HW], BF16, tag="xb")
                for kc in range(KC1):
                    if b == 0:
                        nc.sync.dma_start(out=w1T[:, kc, :], in_=w1T_d[kc])
                    nc.sync.dma_start(out=xb[:, kc, :], in_=x16_d[b, kc])

                # ---- conv1: h1 = relu(x @ w1' + b1) ----
                h1 = actp.tile([P, PT, HW], BF16, tag="h1")
                for mc in range(PT):
                    ps = pmm.tile([P, HW], F32, tag="mm")
                    for kc in range(KC1):
                        for n in range(NHALF):
                            nc.tensor.matmul(
                                ps[:, _ns(n)],
                                w1T[:, kc, mc * P:(mc + 1) * P],
                                xb[:, kc, _ns(n)],
                                start=(kc == 0), stop=(kc == KC1 - 1),
                            )
                    nc.scalar.activation(
                        out=h1[:, mc, :], in_=ps,
                        func=mybir.ActivationFunctionType.Relu,
                        bias=b1[:, mc], scale=1.0,
                    )

                # ---- q/k/v convs ----
                q = actp.tile([P, PT, HW], BF16, tag="q")
                for wT, bias, relu, dest in (
                    (wqT, bq, True, None),
                    (wkT, bk, True, kpad),
                    (wvT, bv, False, vpad),
                ):
                    for mc in range(PT):
                        ps = pmm.tile([P, HW], F32, tag="mm")
                        for kc in range(PT):
                            for n in range(NHALF):
                                nc.tensor.matmul(
                                    ps[:, _ns(n)],
                                    wT[:, kc, mc * P:(mc + 1) * P],
                                    h1[:, kc, _ns(n)],
                                    start=(kc == 0), stop=(kc == PT - 1),
                                )
                        if dest is None:
                            o, i = q[:, mc, :], ps[:]
                        else:
                            o = dest[:, mc, 1:H + 1, 1:W + 1]
                            i = ps.rearrange("p (a b) -> p a b", a=H)
                        nc.scalar.activation(
                            out=o, in_=i,
                            func=(mybir.ActivationFunctionType.Relu if relu
                                  else mybir.ActivationFunctionType.Identity),
                            bias=bias[:, mc], scale=1.0,
                        )


                # ---- attention logits (packed), exp, denominator ----
                # packed tile t rows: 32*(kk%4) + g  for kk in {4t..4t+3}
                epks = []
                den = attnp.tile([HEADS, HW], F32, tag="den")
                denp = paccp.tile([HEADS, HW], F32, tag="acc")
                for t in range(NT):
                    nsh = 4 if t < 2 else 1
                    rows = 32 * nsh
                    Lpk = pLp.tile([P, HW], F32, tag="Lpk")
                    # qpos term: all rows at once per pt chunk
                    for n in range(NHALF):
                        for pt in range(PT):
                            nc.tensor.matmul(
                                Lpk[:rows, _ns(n)],
                                p2[:, pt, t, :rows],
                                q[:, pt, _ns(n)],
                                start=(pt == 0), stop=False,
                                skip_group_check=True,
                            )
                    # qk products + col-tiled group reduce
                    for j in range(nsh):
                        kk = 4 * t + j
                        di, dj = kk // KS, kk % KS
                        for pt in range(PT):
                            tmp = tmpp.tile([P, HW], BF16, tag="tmp")
                            nc.vector.tensor_tensor(
                                out=tmp.rearrange("p (a b) -> p a b", a=H),
                                in0=kpad[:, pt, di:di + H, dj:dj + W],
                                in1=q[:, pt, :].rearrange("p (a b) -> p a b", a=H),
                                op=mybir.AluOpType.mult,
                            )
                            for n in range(NHALF):
                                nc.tensor.matmul(
                                    Lpk[32 * j:32 * (j + 1), _ns(n)],
                                    sel[:, pt, :],
                                    tmp[:, _ns(n)],
                                    start=False, stop=(pt == PT - 1),
                                    tile_position=(0, 32 * j),
                                    skip_group_check=True,
                                )
                    epk = epkp.tile([P, HW], BF16, tag="epk")
                    nc.scalar.activation(
                        out=epk[:rows, :], in_=Lpk[:rows, :],
                        func=mybir.ActivationFunctionType.Exp,
                    )
                    epks.append(epk)
                    # denominator accumulation
                    lhs = sab if t < 2 else eye32
                    for n in range(NHALF):
                        nc.tensor.matmul(
                            denp[:, _ns(n)], lhs[:rows, :], epk[:rows, _ns(n)],
                            start=(t == 0), stop=(t == NT - 1),
                            skip_group_check=True,
                        )
                nc.vector.reciprocal_approx_fast(out=den, in_=denp)

                # recip broadcast head -> channels via DMA
                recip_bc = attnp.tile([P, PT, HW], F32, tag="recip_bc")
                for mc in range(PT):
                    head_bcast_dma(recip_bc[:, mc, :], den[16 * mc:16 * (mc + 1), :])

                # ---- v side: out_pre[c] = sum_kk e_bc * v_shift ----
                h2 = actp.tile([P, PT, HW], BF16, tag="h2")
                for mc in range(PT):
                    acc = paccp.tile([P, HW], F32, tag="acc")
                    for kk in range(NKK):
                        t, j = kk // 4, kk % 4
                        di, dj = kk // KS, kk % KS
                        eb = ebcp.tile([P, HW], BF16, tag="ebc")
                        r0 = 32 * j + 16 * mc
                        head_bcast_dma(eb, epks[t][r0:r0 + 16, :])
                        t2 = tmp2p.tile([P, HW], BF16, tag="tmp2")
                        nc.vector.tensor_tensor(
                            out=t2.rearrange("p (a b) -> p a b", a=H),
                            in0=eb.rearrange("p (a b) -> p a b", a=H),
                            in1=vpad[:, mc, di:di + H, dj:dj + W],
                            op=mybir.AluOpType.mult,
                        )
                        for n in range(NHALF):
                            nc.tensor.matmul(
                                acc[:, _ns(n)], ident, t2[:, _ns(n)],
                                start=(kk == 0), stop=(kk == NKK - 1),
                                skip_group_check=True,
                            )
                    # h2 = relu(acc * recip_bc + batt)
                    t3 = tmp2p.tile([P, HW], F32, tag="t3")
                    nc.vector.tensor_tensor(
                        out=t3, in0=acc, in1=recip_bc[:, mc, :],
                        op=mybir.AluOpType.mult,
                    )
                    nc.scalar.activation(
                        out=h2[:, mc, :], in_=t3,
                        func=mybir.ActivationFunctionType.Relu,
                        bias=batt[:, mc], scale=1.0,
                    )

                # ---- conv3 + residual (identity matmul on bf16 x) + relu ----
                for oc in range(OC):
                    ps = pmm.tile([P, HW], F32, tag="mm")
                    for n in range(NHALF):
                        for kc in range(PT):
                            nc.tensor.matmul(
                                ps[:, _ns(n)],
                                w3T[:, kc, oc * P:(oc + 1) * P],
                                h2[:, kc, _ns(n)],
                                start=(kc == 0), stop=False,
                                skip_group_check=True,
                            )
                        nc.tensor.matmul(
                            ps[:, _ns(n)], ident, xb[:, oc, _ns(n)],
                            start=False, stop=True,
                            skip_group_check=True,
                        )
                    zr = outzp.tile([P, HW], F32, tag="outzr")
                    nc.scalar.activation(
                        out=zr, in_=ps, func=mybir.ActivationFunctionType.Relu,
                        bias=b3[:, oc], scale=1.0,
                    )
                    nc.sync.dma_start(out=out_d[b, oc], in_=zr)

    nc.compile()
    return nc


_PROG = None


def _host_prep(inputs):
    import ml_dtypes
    bf = ml_dtypes.bfloat16
    f = lambda a: np.asarray(a, dtype=np.float32)
    x = f(inputs["x"])
    # fold bn scales into weights (bn(conv(x,W),s,b) = conv(x, s*W) + b)
    w1 = f(inputs["w_conv1"]) * f(inputs["bn1_s"])[:, None]
    wq = f(inputs["wq"]) * f(inputs["bnq_s"])[:, None]
    wk = f(inputs["wk"]) * f(inputs["bnk_s"])[:, None]
    # fold bnatt scale through the (linear) attention-value path into v
    sv = f(inputs["bnatt_s"]) * f(inputs["bnv_s"])
    wv = f(inputs["wv"]) * sv[:, None]
    bv = f(inputs["bnatt_s"]) * f(inputs["bnv_b"])
    w3 = f(inputs["w_conv3"]) * f(inputs["bn3_s"])[:, None]

    posf = (f(inputs["pos_h"]) + f(inputs["pos_w"])).reshape(WIDTH, NKK)

    sel = np.zeros((PT, P, HEADS), np.float32)
    for pt in range(P // 64):
        pass
    for pt in range(PT):
        for c in range(P):
            sel[pt, c, pt * (P // D) + c // D] = 1.0
    # p2[pt, c, t, 32*j+g] = pos[c_global, 4t+j] if head(c_global)==g
    p2 = np.zeros((PT, P, NT, P), np.float32)
    for pt in range(PT):
        for c in range(P):
            g = pt * (P // D) + c // D
            for kk in range(NKK):
                t, j = kk // 4, kk % 4
                p2[pt, c, t, 32 * j + g] = posf[pt * P + c, kk]
    # sab[r, g] = 1 if r % 32 == g (sum over the 4 packed kk rows)
    sab = np.zeros((P, HEADS), np.float32)
    for r in range(P):
        sab[r, r % HEADS] = 1.0
    com = {
        "w1T": np.ascontiguousarray(w1.T.reshape(KC1, P, WIDTH)).astype(bf),
        "wqT": np.ascontiguousarray(wq.T.reshape(PT, P, WIDTH)).astype(bf),
        "wkT": np.ascontiguousarray(wk.T.reshape(PT, P, WIDTH)).astype(bf),
        "wvT": np.ascontiguousarray(wv.T.reshape(PT, P, WIDTH)).astype(bf),
        "w3T": np.ascontiguousarray(w3.T.reshape(PT, P, OUT)).astype(bf),
        "b1": f(inputs["bn1_b"]).reshape(PT, P, 1),
        "bq": f(inputs["bnq_b"]).reshape(PT, P, 1),
        "bk": f(inputs["bnk_b"]).reshape(PT, P, 1),
        "bv": bv.reshape(PT, P, 1),
        "batt": f(inputs["bnatt_b"]).reshape(PT, P, 1),
        "b3": f(inputs["bn3_b"]).reshape(OC, P, 1),
        "sel": sel.astype(bf),
        "p2": p2.astype(bf),
        "sab": sab.astype(bf),
        "eye32": np.eye(HEADS, dtype=np.float32).astype(bf),
        "ident": np.eye(P, dtype=np.float32).astype(bf),
    }
    xr = x.reshape(B, KC1, P, HW)
    in_maps = []
    for c in range(NC_):
        xs = np.ascontiguousarray(xr[c * BL:(c + 1) * BL])
        in_maps.append(dict(com, x16=xs.astype(bf)))
    return in_maps


def kernel(**inputs):
    global _PROG
    if _PROG is None:
        _PROG = build_program()
    in_maps = _host_prep(inputs)
    res = run_bass_kernel_spmd(_PROG, in_maps, core_ids=list(range(NC_)))
    outs = [res.results[c]["out"].reshape(BL, OUT, H, W) for c in range(NC_)]
    return np.concatenate(outs, axis=0)
